# revision 13
# baseline (speedup 1.0000x reference)
"""Bahdanau attention Trainium2 kernel (8-core data-parallel over batch).

reference:
    proj_v = values @ W1 + b1            # [B,T,U]
    proj_q = (query @ W2 + b2)[:,None,:] # [B,1,U]
    score  = tanh(proj_v + proj_q) @ V + bV
    score += mask * -1e9
    attn   = softmax(score, axis=1)      # [B,T,1]
    ctx    = sum(attn * values, axis=1)  # [B,D]
    return (ctx, attn)

Sharding: batch dim across 8 cores (4 batches each); weights replicated.
bV dropped (softmax is shift-invariant).

Per-core plan (layout B: proj_v^T tiles [U=128p, T=512f] in PSUM):
  - values chunks land in SBUF through a block-rearranged DMA (32-elem
    runs) + DVE 32x32 stream-transpose (f32) + DVE rounding copy to f32r,
    giving values^T tiles [D=128p, 512f] for the PE.
  - GEMM: lhsT = W1 128x128 blocks (natural layout, bitcast-f32r DMA),
    rhs = values^T, fp32r (full-rate fp32 matmul), accumulate over D.
  - ScalarE: tanh(psum + (proj_q+b1+b2) per-partition bias) in one op,
    output rounded to f32r.
  - score chunk [1,512] = sum_u V_u^T @ tanh_u on PE (fp32r).
  - masked softmax along the free axis on partition 0.
  - phase 2: context[b] = w @ values[b] on PE with w tiles [128,1]
    (stride-16 interleaved T order) as stationary operand and
    naturally-loaded values tiles as rhs (bitcast f32r).
"""

import numpy as np

import concourse.bacc as bacc
import concourse.bass as bass
import concourse.mybir as mybir
from concourse.bass import ts
from concourse.tile import TileContext

B, T, D, U = 32, 2048, 1024, 1024
NCORES = 8
BC = B // NCORES          # batches per core
P = 128
TC = 512                  # T chunk (max fp32 moving free dim)
NT = T // TC              # 4 chunks per batch
KD = D // P               # 8 contraction tiles
NU = U // P               # 8 unit tiles
NJ = T // P               # 16 T tiles for phase 2

F32 = mybir.dt.float32
F32R = mybir.dt.float32r

_CACHED = {}


def build_module():
    nc = bacc.Bacc("TRN2")

    values = nc.dram_tensor("values", [BC, T, D], F32, kind="ExternalInput")
    query = nc.dram_tensor("query", [BC, D], F32, kind="ExternalInput")
    mask = nc.dram_tensor("mask", [BC, T], F32, kind="ExternalInput")
    W1 = nc.dram_tensor("W1", [D, U], F32, kind="ExternalInput")
    b1 = nc.dram_tensor("b1", [U], F32, kind="ExternalInput")
    W2 = nc.dram_tensor("W2", [D, U], F32, kind="ExternalInput")
    b2 = nc.dram_tensor("b2", [U], F32, kind="ExternalInput")
    V = nc.dram_tensor("V", [U, 1], F32, kind="ExternalInput")
    ctx_out = nc.dram_tensor("context", [BC, D], F32, kind="ExternalOutput")
    attn_out = nc.dram_tensor("attn", [BC, T], F32, kind="ExternalOutput")

    with TileContext(nc) as tc, \
         tc.tile_pool(name="consts", bufs=1) as consts, \
         tc.tile_pool(name="small", bufs=8) as small:
        # ---- constants (bitcast-f32r DMA loads are legal PE operands) ----
        w1r = consts.tile([P, KD, U], F32R)
        nc.sync.dma_start(
            out=w1r, in_=W1.rearrange("(k p) u -> p k u", p=P).bitcast(F32R)
        )
        v_r = consts.tile([P, NU], F32R)
        nc.sync.dma_start(
            out=v_r, in_=V.rearrange("(u p) one -> p (u one)", p=P).bitcast(F32R)
        )
        b1_sb = consts.tile([P, NU], F32)
        nc.sync.dma_start(out=b1_sb, in_=b1.rearrange("(u p) -> p u", p=P))
        b2_sb = consts.tile([P, NU], F32)
        nc.sync.dma_start(out=b2_sb, in_=b2.rearrange("(u p) -> p u", p=P))
        b12 = consts.tile([P, NU], F32)
        nc.vector.tensor_add(b12, b1_sb, b2_sb)

        # ---- proj_q^T: pqb[p,u,b] = (query[b] @ W2 + b1 + b2)[128u+p] ----
        pqb = consts.tile([P, NU, BC], F32)
        with (
            tc.tile_pool(name="pre", bufs=2) as pre,
            tc.tile_pool(name="pre_ps", bufs=2, space="PSUM") as pre_ps,
        ):
            qT = pre.tile([P, KD, BC], F32)
            for bq in range(BC):
                nc.sync.dma_start(
                    out=qT[:, :, bq],
                    in_=query[bq].rearrange("(k p) -> p k", p=P),
                )
            for u in range(NU):
                w2t = pre.tile([P, KD, P], F32, tag="w2t")
                nc.sync.dma_start(
                    out=w2t,
                    in_=W2[:, ts(u, P)].rearrange("(k p) uf -> p k uf", p=P),
                )
                psq = pre_ps.tile([P, BC], F32)
                for k in range(KD):
                    nc.tensor.matmul(
                        psq, lhsT=w2t[:, k], rhs=qT[:, k],
                        start=(k == 0), stop=(k == KD - 1),
                    )
                nc.vector.tensor_scalar_add(pqb[:, u], psq, b12[:, u : u + 1])

        with (
            tc.tile_pool(name="stage", bufs=2) as stage,
            tc.tile_pool(name="vtf", bufs=3) as vtf,
            tc.tile_pool(name="vt", bufs=2 * KD) as vtp,
            tc.tile_pool(name="tanh", bufs=4) as thp,
            tc.tile_pool(name="gemm_ps", bufs=3, space="PSUM") as gps,
            tc.tile_pool(name="score_ps", bufs=2, space="PSUM") as sps,
            tc.tile_pool(name="ctx_ps", bufs=1, space="PSUM") as cps,
            tc.tile_pool(name="score_sb", bufs=1) as ssb,
            tc.tile_pool(name="w_sb", bufs=1) as wsb,
            tc.tile_pool(name="maskp", bufs=2) as maskp,
            tc.tile_pool(name="ctxsb", bufs=2) as ctxp,
            tc.tile_pool(name="wdram", bufs=2, space="DRAM") as wdram,
            tc.tile_pool(name="vnat", bufs=3) as vnat,
        ):
            # ---- main loop over batches ----
            for b in range(BC):
                score_sb = ssb.tile([1, T], F32)
                for c in range(NT):
                    # staged load: S[32bi+p, k, 32bj+q] =
                    #   values[b, c*TC+32bj+p, 128k+32bi+q]
                    S = stage.tile([P, KD, TC], F32)
                    vb = values[b, c * TC : (c + 1) * TC, :]
                    for bi in range(4):
                        for k in range(KD):
                            src_ap = bass.AP(
                                tensor=vb.tensor,
                                offset=vb.offset + 32 * bi + P * k,
                                # (p, bj, q): elem = (32bj+p)*D + 128k+32bi+q
                                ap=[[D, 32], [32 * D, TC // 32], [1, 32]],
                            )
                            nc.sync.dma_start(
                                out=S[32 * bi : 32 * (bi + 1), k], in_=src_ap
                            )
                    # DVE 32x32 block transpose (f32) -> rounding copy to f32r
                    vt = []
                    for k in range(KD):
                        vtk_f = vtf.tile([P, TC], F32, tag="vtf")
                        nc.vector.transpose(out=vtk_f, in_=S[:, k])
                        vtk = vtp.tile([P, TC], F32R, tag="vt")
                        nc.vector.tensor_copy(vtk, vtk_f)
                        vt.append(vtk)

                    score_ps = sps.tile([1, TC], F32)
                    for u in range(NU):
                        ps = gps.tile([P, TC], F32)
                        for k in range(KD):
                            nc.tensor.matmul(
                                ps,
                                lhsT=w1r[:, k, ts(u, P)],
                                rhs=vt[k][:],
                                start=(k == 0),
                                stop=(k == KD - 1),
                            )
                        th = thp.tile([P, TC], F32R)
                        nc.scalar.activation(
                            th, ps, mybir.ActivationFunctionType.Tanh,
                            bias=pqb[:, u, b : b + 1],
                        )
                        nc.tensor.matmul(
                            score_ps,
                            lhsT=v_r[:, u : u + 1],
                            rhs=th[:],
                            start=(u == 0),
                            stop=(u == NU - 1),
                        )
                    # raw score chunk -> SBUF
                    nc.vector.tensor_copy(
                        score_sb[0:1, c * TC : (c + 1) * TC], score_ps
                    )

                # ---- mask + softmax over free axis (partition 0) ----
                mk = maskp.tile([1, T], F32, tag="mask")
                nc.sync.dma_start(out=mk, in_=mask[b : b + 1, :])
                score_m = maskp.tile([1, T], F32, tag="score_m")
                nc.vector.scalar_tensor_tensor(
                    out=score_m, in0=mk, scalar=-1.0e9, in1=score_sb,
                    op0=mybir.AluOpType.mult, op1=mybir.AluOpType.add,
                )
                m = small.tile([1, 1], F32, tag="m")
                nc.vector.reduce_max(m, score_m, axis=mybir.AxisListType.X)
                negm = small.tile([1, 1], F32, tag="negm")
                nc.vector.tensor_scalar_mul(negm, m, -1.0)
                esum = small.tile([1, 1], F32, tag="esum")
                expv = wsb.tile([1, T], F32, tag="expv")
                nc.scalar.activation(
                    expv, score_m, mybir.ActivationFunctionType.Exp,
                    bias=negm, accum_out=esum,
                )
                rsum = small.tile([1, 1], F32, tag="rsum")
                nc.vector.reciprocal(rsum, esum)
                w_f = wsb.tile([1, T], F32, tag="w_f")
                nc.vector.tensor_scalar_mul(w_f, expv, rsum)
                w_r = wsb.tile([1, T], F32R, tag="w_r")
                nc.vector.tensor_scalar_mul(w_r, expv, rsum)

                # attention weights out + transposed copy for phase 2
                nc.sync.dma_start(out=attn_out[b : b + 1, :], in_=w_f)
                wdr = wdram.tile([1, T], F32R)
                nc.sync.dma_start(out=wdr, in_=w_r)
                # wT[p, g] = w[16p + g]; phase-2 tile g covers t = g + 16*p
                wT = small.tile([P, NJ], F32R, tag="wT")
                nc.sync.dma_start(
                    out=wT, in_=wdr[:].rearrange("one (p g) -> (one p) g", p=P)
                )

                # ---- phase 2: context[b] = w @ values[b] ----
                ctx_ps = cps.tile([1, D], F32)
                for g in range(NJ):
                    vn = vnat.tile([P, D], F32R)
                    # rows t = g, g+16, ..., g+16*127
                    nc.sync.dma_start(
                        out=vn, in_=values[b, g :: NJ, :].bitcast(F32R)
                    )
                    for n in range(2):
                        nc.tensor.matmul(
                            ctx_ps[:, ts(n, TC)],
                            lhsT=wT[:, g : g + 1],
                            rhs=vn[:, ts(n, TC)],
                            start=(g == 0),
                            stop=(g == NJ - 1),
                        )
                ctx_sb = ctxp.tile([1, D], F32, tag="ctx_sb")
                nc.scalar.copy(ctx_sb, ctx_ps)
                nc.sync.dma_start(out=ctx_out[b : b + 1, :], in_=ctx_sb)

    nc.compile()
    return nc


def _get_module():
    if "nc" not in _CACHED:
        _CACHED["nc"] = build_module()
    return _CACHED["nc"]


def kernel(values, query, mask, W1, b1, W2, b2, V, bV):
    from concourse.bass_utils import run_bass_kernel_spmd

    values = np.asarray(values, dtype=np.float32)
    query = np.asarray(query, dtype=np.float32)
    mask = np.asarray(mask, dtype=np.float32)
    common = {
        "W1": np.asarray(W1, dtype=np.float32),
        "b1": np.asarray(b1, dtype=np.float32),
        "W2": np.asarray(W2, dtype=np.float32),
        "b2": np.asarray(b2, dtype=np.float32),
        "V": np.asarray(V, dtype=np.float32),
    }
    in_maps = []
    for i in range(NCORES):
        s = slice(i * BC, (i + 1) * BC)
        in_maps.append(
            {"values": values[s], "query": query[s], "mask": mask[s], **common}
        )

    nc = _get_module()
    res = run_bass_kernel_spmd(nc, in_maps, core_ids=list(range(NCORES)))
    context = np.concatenate([r["context"] for r in res.results], axis=0)
    attn = np.concatenate([r["attn"] for r in res.results], axis=0)
    return context.astype(np.float32), attn[:, :, None].astype(np.float32)
